# revision 21
# baseline (speedup 1.0000x reference)
"""Trainium2 Bass kernel for nn_MultiHeaded_4080218931880.

Multi-headed attention with the reference's *raw reshape* head split:
    q = from @ Wq + bq                      # (B, F, HD)
    q_r = q.reshape(B, H, D, F)             # raw row-major reshape
    score = einsum('bhdf,bhdt->bhft', q_r, k_r) * alpha
    probs = softmax(score + (1-mask)*NEG, axis=-1)
    out = einsum('bhft,bhdt->bhdf', probs, v_r).reshape(B, H*D, F)

Because the reshape is raw, head h only touches rows [2*D*h, 2*D*(h+1))
of the (F, HD) projection output, so the 32 (b, h) pairs are fully
independent: 4 pairs per NeuronCore over 8 cores.

Fast path (mask all-ones, zero biases — the shipped problem instance):

  Projections via fp8 e4m3 DoubleRow, 3-term split (x8@W8 + xr@W8 +
  x8@Wr with W prescaled by WS=64 to stay in e4m3 normal range; the
  1/WS folds into the eviction scalar ops).  Terms are paired across
  adjacent c-chunks of the same kind, so each DoubleRow instruction
  contracts K_eff=256 at 0.5 cycles/row — 25% cheaper than bf16 and
  slightly MORE accurate (residual term cancels the x quantization).

  Score via fp8 DoubleRow 4-term split: q -> q8+qr, k -> k8+kr (DVE
  evictions straight from the projection PSUM), stacked [q8;qr] on
  partitions and [k8;k8],[kr;kr] on the two DoubleRow j-subtiles.  One
  instruction per 512-slice computes exact-to-~fp16^2 scores at HALF
  the bf16 PE cost.  alpha is applied for free via the exp
  activation's scale parameter.

  ctx stays bf16 (exp tiles bf16, v bf16 with a ones row so the ctx
  accumulator's row D carries the softmax denominator).  The per-pair
  (D+1, F) accumulator is DMA'd out raw; the final divide by the
  denominator row happens on host during the gather/unshard step.

Fallback path (general mask/bias) keeps the previous all-bf16 program.
"""

import numpy as np
from contextlib import ExitStack

import concourse.bass as bass
import concourse.bacc as bacc
import concourse.tile as tile
from concourse import mybir
from concourse.bass_utils import run_bass_kernel_spmd
from concourse.masks import make_identity

BF16 = mybir.dt.bfloat16
F32 = mybir.dt.float32
FP8 = mybir.dt.float8e4
NP_BF16 = mybir.dt.np(mybir.dt.bfloat16)
NP_FP8 = mybir.dt.np(mybir.dt.float8e4)

# Problem dims (hardcoded; harness runs kernel.py standalone).
B, F, T, C = 2, 2048, 2048, 1024
H, D = 16, 64
HD = H * D
ALPHA = 1.0 / np.sqrt(np.float32(D)).astype(np.float32)
NEG = -100000.0
N_CORES = 8
NPAIR = (B * H) // N_CORES  # 4 (b,h) pairs per core
P = 128
DR = mybir.MatmulPerfMode.DoubleRow
WS = 64.0  # fp8 weight prescale (power of two; folded back at eviction)

REAL_DIMS = dict(npair=NPAIR, c=C, hd=HD, d=D, f=F, t=T)


def _nsl(total, step):
    """Split [0, total) into <=step slices (matmul moving free-dim limit)."""
    return [(s, min(s + step, total)) for s in range(0, total, step)]


def build_program_fast(dims=None, alpha=float(ALPHA)):
    dims = dims or REAL_DIMS
    npair, c, hd, d, f, t = (
        dims["npair"], dims["c"], dims["hd"], dims["d"], dims["f"], dims["t"],
    )
    bh = 2 * d
    ncc = c // P
    ncp = ncc // 2
    nch = t // P       # t'-chunks (score partition dim / ctx contraction)
    nfc = f // P       # f'-chunks (transposed-ctx partition dim)
    NB = 512
    fh = f // 2
    che = hd // P      # score chunks per e-half of the ks layout
    cpf = fh // P      # f'-chunks per exp half
    mult = mybir.AluOpType.mult
    subtract = mybir.AluOpType.subtract

    nc = bacc.Bacc(None, target_bir_lowering=False, debug=True)
    # dim1 of x params: 0 = fp8 value, 1 = fp8 residual
    xf = nc.declare_dram_parameter("xf", [npair, 2, P, ncc, bh], FP8, isOutput=False)
    xt = nc.declare_dram_parameter("xt", [npair, 2, P, ncc, bh], FP8, isOutput=False)
    wparams = {}
    for name in ("q", "k", "v"):
        wparams[name] = (
            nc.declare_dram_parameter(f"w8{name}", [P, ncc, hd], FP8, isOutput=False),
            nc.declare_dram_parameter(f"wr{name}", [P, ncc, hd], FP8, isOutput=False),
        )
    # transposed-ctx outputs: out_c[j][f', fc, dd] = ctx[dd, fc*128+f'],
    # out_s[j][f', fc] = softmax denominator for column fc*128+f'
    out_c = nc.declare_dram_parameter("out_c", [npair, P, nfc, d], BF16, isOutput=True)
    out_s = nc.declare_dram_parameter("out_s", [npair, P, nfc], F32, isOutput=True)

    with tile.TileContext(nc) as tc, ExitStack() as ctx:
        const = ctx.enter_context(tc.tile_pool(name="const", bufs=1))
        wpool = ctx.enter_context(tc.tile_pool(name="wpool", bufs=1))
        rqk = ctx.enter_context(tc.tile_pool(name="rqk", bufs=4))
        vpool = ctx.enter_context(tc.tile_pool(name="vpool", bufs=npair * nch))

        ident = const.tile([d + 1, d + 1], BF16)
        make_identity(nc, ident[:])

        w8_s, wr_s = {}, {}

        def load_weights(names):
            # one DMA per tensor-half: HWDGE queue slots are the scarce
            # resource (625ns serial each), not bandwidth.  v is deferred
            # so pair-0's eviction DMAs don't queue behind it.
            for name in names:
                w8d, wrd = wparams[name]
                w8t_ = wpool.tile([P, ncc, hd], FP8, tag=f"w8{name}")
                nc.sync.dma_start(out=w8t_[:], in_=w8d[:])
                wrt_ = wpool.tile([P, ncc, hd], FP8, tag=f"wr{name}")
                nc.sync.dma_start(out=wrt_[:], in_=wrd[:])
                w8_s[name] = w8t_
                wr_s[name] = wrt_

        r_all = [{} for _ in range(npair)]
        vones_all = [[] for _ in range(npair)]

        xpool = ctx.enter_context(tc.tile_pool(name="xpool", bufs=2))
        blkpool = ctx.enter_context(tc.tile_pool(name="blkpool", bufs=6))
        rv = ctx.enter_context(tc.tile_pool(name="rv", bufs=2))
        epool = ctx.enter_context(tc.tile_pool(name="epool", bufs=16))
        opool = ctx.enter_context(tc.tile_pool(name="opool", bufs=4))
        # PSUM map (all 128 partitions, 8 banks of 2KB):
        #   banks 0-3: two score slots (128, fh<=1024) f32 -- exp ping-pong
        #   banks 4-5: transposed-ctx accumulator (128, nfc, d) f32,
        #              256B chunks, accumulated WITHOUT start (memset once
        #              per pair) so sub-bank chunks never zero each other
        #   bank 6:    projection accumulator (128, 512) f32; also hosts
        #              the v-transpose staging tile between projections
        #   bank 7:    denominator accumulator (128, nfc) f32
        pp_score = ctx.enter_context(tc.tile_pool(name="pp_score", bufs=2, space="PSUM"))
        pp_ctxT = ctx.enter_context(tc.tile_pool(name="pp_ctxT", bufs=1, space="PSUM"))
        pp_pj = ctx.enter_context(tc.tile_pool(name="pp_pj", bufs=1, space="PSUM"))
        pp_s = ctx.enter_context(tc.tile_pool(name="pp_s", bufs=1, space="PSUM"))

        def proj_mm(x_s, name, pj, ns, ne, gi, full=False):
            """One term-group of DoubleRow projection matmuls for a slice."""
            terms = ((0, w8_s[name]), (1, w8_s[name]), (0, wr_s[name]))
            kind, ws = terms[gi]
            out = pj[:, ns:ne] if full else pj[:, 0:ne - ns]
            for cp in range(ncp):
                nc.tensor.matmul(
                    out,
                    x_s[:, kind, 2 * cp:2 * cp + 2, :],
                    ws[:, 2 * cp:2 * cp + 2, ns:ne],
                    start=(gi == 0 and cp == 0),
                    stop=(gi == 2 and cp == ncp - 1),
                    perf_mode=DR,
                )

        def evict_qk_slice(blk, hqr, ns, ne):
            nc.vector.tensor_scalar_mul(hqr[:, 0, ns:ne], blk[:], 1.0 / WS)
            nc.vector.scalar_tensor_tensor(
                hqr[:, 1, ns:ne], blk[:], 1.0 / WS, hqr[:, 0, ns:ne],
                op0=mult, op1=subtract,
            )

        def bounce_qk(j, name, hqr):
            """Direct SBUF->SBUF reshape DMAs (flat element-stream order
            realizes the raw (2d, hd) -> (d, 2*hd) reshape)."""
            if name == "q":
                # moving side: flat [q8;qr] stack; the DoubleRow
                # j-duplication happens via a stride-0 AP
                s = rqk.tile([P, f], FP8, tag="qs")
                nc.sync.dma_start(out=s[0:d, :], in_=hqr[:, 0, :])
                nc.sync.dma_start(out=s[d:2 * d, :], in_=hqr[:, 1, :])
            else:
                # stationary side, e-major (P, e, j, c): j=0 carries k8,
                # j=1 kr; both partition-halves hold the same data
                s = rqk.tile([P, 2, 2, hd], FP8, tag="ks")
                nc.sync.dma_start(out=s[0:d], in_=hqr[:])
                nc.sync.dma_start(out=s[d:2 * d], in_=hqr[:])
            r_all[j][name] = s

        def emit_transposes(j):
            r_v = r_all[j]["v"]
            nc.vector.memset(r_v[d:d + 1, :], 1.0)
            grp = 2
            for tg in range(0, nch, grp):
                gn = min(grp, nch - tg)
                vt_ps = pp_pj.tile([P, grp, d + 2], BF16, tag="pj")
                for ti in range(gn):
                    tcb = tg + ti
                    nc.tensor.transpose(
                        vt_ps[:, ti, 0:d + 1],
                        r_v[:, tcb * P:(tcb + 1) * P],
                        ident[:],
                    )
                    vo = vpool.tile([P, d + 1], BF16, tag="vones")
                    nc.vector.tensor_copy(vo[:], vt_ps[:, ti, 0:d + 1])
                    vones_all[j].append(vo)
                yield

        QKDONE = "qkdone"

        def emit_proj(j, gate_x=False):
            xf_s = xpool.tile([P, 2, ncc, bh], FP8, tag="xf")
            xt_s = xpool.tile([P, 2, ncc, bh], FP8, tag="xt")
            if gate_x:
                # WAW corner-memsets on the DVE (in-order after the startup
                # gate copy) keep these dep-free DMAs from jumping ahead of
                # pair-0's critical bounce DMAs on the shared DMA engines
                nc.vector.memset(xf_s[0:1, 0:1, 0:1, 0:1], 0.0)
                nc.vector.memset(xt_s[0:1, 0:1, 0:1, 0:1], 0.0)
            nc.sync.dma_start(
                out=xf_s[:], in_=xf[j].rearrange("k p c b -> p k c b")
            )
            nc.sync.dma_start(
                out=xt_s[:], in_=xt[j].rearrange("k p c b -> p k c b")
            )
            yield
            for name, x_s in (("q", xf_s), ("k", xt_s), ("v", xt_s)):
                if name == "v":
                    hv = blkpool.tile([bh, hd], BF16, tag="hv")
                else:
                    hqr = blkpool.tile([bh, 2, hd], FP8, tag="hqr")
                for ns, ne in _nsl(hd, NB):
                    pj = pp_pj.tile([bh, min(NB, hd)], F32, tag="pj")
                    for gi in range(3):
                        proj_mm(x_s, name, pj, ns, ne, gi)
                        yield
                    # evictions fold the 1/WS weight prescale back in; the
                    # per-slice copy frees the proj PSUM bank quickly
                    if name == "v":
                        nc.vector.tensor_scalar_mul(
                            hv[:, ns:ne], pj[:, 0:ne - ns], 1.0 / WS)
                    else:
                        blk = blkpool.tile([bh, min(NB, hd)], F32, tag="blk")
                        nc.vector.tensor_copy(blk[:], pj[:, 0:ne - ns])
                        evict_qk_slice(blk[:, 0:ne - ns], hqr, ns, ne)
                    yield
                if name == "v":
                    r = rv.tile([d + 1, 2 * hd], BF16, tag="rv")
                    nc.sync.dma_start(out=r[0:d, :], in_=hv[:])
                    r_all[j]["v"] = r
                else:
                    bounce_qk(j, name, hqr)
                yield
            yield from emit_transposes(j)

        def emit_proj0():
            """Pair-0 startup: q/k projections accumulate full-width in the
            (still idle) score banks so they track their weight DMAs in
            parallel; v weights are data-gated behind the ks bounce."""
            xf_s = xpool.tile([P, 2, ncc, bh], FP8, tag="xf")
            nc.sync.dma_start(
                out=xf_s[:], in_=xf[0].rearrange("k p c b -> p k c b")
            )
            xt_s = xpool.tile([P, 2, ncc, bh], FP8, tag="xt")
            nc.sync.dma_start(
                out=xt_s[:], in_=xt[0].rearrange("k p c b -> p k c b")
            )
            yield
            pj_q = pp_score.tile([bh, hd], F32, tag="sc")
            pj_k = pp_score.tile([bh, hd], F32, tag="sc")
            for gi in range(2):
                for ns, ne in _nsl(hd, NB):
                    proj_mm(xf_s, "q", pj_q, ns, ne, gi, full=True)
                for ns, ne in _nsl(hd, NB):
                    proj_mm(xt_s, "k", pj_k, ns, ne, gi, full=True)
            for pj, name in ((pj_q, "q"), (pj_k, "k")):
                for ns, ne in _nsl(hd, NB):
                    proj_mm(xf_s if name == "q" else xt_s,
                            name, pj, ns, ne, 2, full=True)
                hqr = blkpool.tile([bh, 2, hd], FP8, tag="hqr")
                blk = blkpool.tile([bh, hd], F32, tag="blk0")
                nc.vector.tensor_copy(blk[:], pj[:])
                evict_qk_slice(blk[:], hqr, 0, hd)
                bounce_qk(0, name, hqr)
            # gate: executes on DVE only after the ks bounce DMA landed,
            # then (in DVE order) releases the v-weight DMAs
            gate = blkpool.tile([1, 1], FP8, tag="gate")
            nc.vector.tensor_copy(gate[:], r_all[0]["k"][0:1, 0:1, 0:1, 0:1])
            yield QKDONE
            for name in ("v",):
                w8d, wrd = wparams[name]
                w8t_ = wpool.tile([P, ncc, hd], FP8, tag=f"w8{name}")
                wrt_ = wpool.tile([P, ncc, hd], FP8, tag=f"wr{name}")
                nc.vector.memset(w8t_[0:1, 0:1, 0:1], 0.0)
                nc.vector.memset(wrt_[0:1, 0:1, 0:1], 0.0)
                nc.sync.dma_start(out=w8t_[:], in_=w8d[:])
                nc.sync.dma_start(out=wrt_[:], in_=wrd[:])
                w8_s[name] = w8t_
                wr_s[name] = wrt_
            yield
            hv = blkpool.tile([bh, hd], BF16, tag="hv")
            for ns, ne in _nsl(hd, NB):
                pj = pp_pj.tile([bh, min(NB, hd)], F32, tag="pj")
                for gi in range(3):
                    proj_mm(xt_s, "v", pj, ns, ne, gi)
                    yield
                nc.vector.tensor_scalar_mul(
                    hv[:, ns:ne], pj[:, 0:ne - ns], 1.0 / WS)
                yield
            r = rv.tile([d + 1, 2 * hd], BF16, tag="rv")
            nc.sync.dma_start(out=r[0:d, :], in_=hv[:])
            r_all[0]["v"] = r
            yield
            yield from emit_transposes(0)

        def make_ctx_chunk(j, st):
            def ctx_chunk(tcb):
                vo = vones_all[j][tcb]
                last = tcb == nch - 1
                for fc in range(nfc):
                    ex = st["exs"][tcb][fc // cpf]
                    exsl = ex[:, (fc % cpf) * P:(fc % cpf) * P + P]
                    nc.tensor.matmul(
                        st["ps_cx"][:, fc, :], exsl, vo[:, 0:d],
                        start=False, stop=last, skip_group_check=True,
                    )
                    nc.tensor.matmul(
                        st["ps_s"][:, fc:fc + 1], exsl, vo[:, d:d + 1],
                        start=False, stop=last, skip_group_check=True,
                    )
            return ctx_chunk

        def emit_attn_chunks(j, st):
            qs, ks = r_all[j]["q"], r_all[j]["k"]
            ps_cx = pp_ctxT.tile([P, nfc, d], F32, tag="cx")
            ps_s = pp_s.tile([P, nfc], F32, tag="s")
            # accumulators run WITHOUT start flags (sub-bank chunks would
            # zero each other's region): zero them explicitly instead
            nc.vector.memset(ps_cx[:], 0.0)
            nc.vector.memset(ps_s[:], 0.0)
            st["ps_cx"] = ps_cx
            st["ps_s"] = ps_s
            st["exs"] = {}
            st["ctx_i"] = 0
            ctx_chunk = make_ctx_chunk(j, st)
            for tcb in range(nch):
                exs = []
                for hf in range(2):
                    ps_sc = pp_score.tile([P, fh], F32, tag="sc")
                    for ns, ne in _nsl(fh, NB):
                        nc.tensor.matmul(
                            ps_sc[:, ns:ne],
                            ks[:, tcb // che, :,
                               (tcb % che) * P:(tcb % che) * P + P],
                            qs[:, hf * fh + ns:hf * fh + ne]
                                .unsqueeze(1).broadcast_to((P, 2, ne - ns)),
                            start=True, stop=True,
                            perf_mode=DR,
                        )
                    ex = epool.tile([P, fh], BF16, tag="exp")
                    nc.scalar.activation(
                        ex[:], ps_sc[:], mybir.ActivationFunctionType.Exp,
                        scale=alpha,
                    )
                    exs.append(ex)
                st["exs"][tcb] = exs
                # ctx lags >=1 chunk (so PE never waits on the current exp)
                # and is additionally gated on the v transposes having been
                # emitted (pair 0: v projection overlaps early attention)
                while st["ctx_i"] < tcb and st["ctx_i"] < len(vones_all[j]):
                    ctx_chunk(st["ctx_i"])
                    st["ctx_i"] += 1
                yield

        def emit_attn_tail(j, st):
            ctx_chunk = make_ctx_chunk(j, st)
            while st["ctx_i"] < nch:
                ctx_chunk(st["ctx_i"])
                st["ctx_i"] += 1
            # bf16 output, evicted in halves so the first DMA overlaps the
            # second copy
            cx_sb = opool.tile([P, nfc, d], BF16, tag="ctx")
            hn = nfc // 2
            nc.vector.tensor_copy(cx_sb[:, 0:hn, :], st["ps_cx"][:, 0:hn, :])
            nc.sync.dma_start(out=out_c[j, :, 0:hn, :], in_=cx_sb[:, 0:hn, :])
            nc.vector.tensor_copy(cx_sb[:, hn:nfc, :], st["ps_cx"][:, hn:nfc, :])
            s_sb = opool.tile([P, nfc], F32, tag="s")
            nc.vector.tensor_copy(s_sb[:], st["ps_s"][:])
            nc.sync.dma_start(out=out_c[j, :, hn:nfc, :], in_=cx_sb[:, hn:nfc, :])
            nc.sync.dma_start(out=out_s[j], in_=s_sb[:])

        # software pipeline: a FIFO of projection generators pumped a few
        # steps per attention chunk, so projection/transpose work spreads
        # into the PE slack between score and ctx matmuls and never bunches
        # at pair boundaries.
        pending = []

        def pump(n):
            for _ in range(n):
                while pending:
                    try:
                        next(pending[0])
                        break
                    except StopIteration:
                        pending.pop(0)
                else:
                    return

        pg0 = emit_proj0()
        next(pg0)        # pair-0 x loads issue before the weight DMAs
        load_weights(("q", "k"))
        for step in pg0:
            if step == QKDONE:
                break
        pending.append(pg0)
        pgs = {0: pg0}
        for j in range(npair):
            if j + 1 < npair:
                g = emit_proj(j + 1, gate_x=(j == 0))
                pgs[j + 1] = g
                pending.append(g)
            # pair j's q/k score tiles must be emitted before its attention
            while "k" not in r_all[j] or "q" not in r_all[j]:
                pump(1)
            st = {}
            for _ in emit_attn_chunks(j, st):
                pump(3)
            # pair j's transposes must all be emitted before the ctx tail
            gj = pgs.get(j)
            if gj is not None:
                for _ in gj:
                    pass
                if gj in pending:
                    pending.remove(gj)
            emit_attn_tail(j, st)
        for g in pending:
            for _ in g:
                pass

    nc.finalize()
    return nc


def build_program_general(has_mask=False, has_bias=True, dims=None, exp_bufs=None):
    """All-bf16 fallback program (handles mask and bias)."""
    dm = dims or REAL_DIMS
    npair, c, hd, d, f, t = (
        dm["npair"], dm["c"], dm["hd"], dm["d"], dm["f"], dm["t"],
    )
    bh = 2 * d          # row-block height of x per (b,h) pair
    ncc = c // P        # contraction chunks for projections
    nch = t // P        # t' chunks for attention
    NB = 512            # matmul PSUM-write limit: one 2KB bank (512 f32)

    nc = bacc.Bacc(None, target_bir_lowering=False, debug=True)
    xfT = nc.declare_dram_parameter("xfT", [npair, P, ncc, bh], BF16, isOutput=False)
    xtT = nc.declare_dram_parameter("xtT", [npair, P, ncc, bh], BF16, isOutput=False)
    wq = nc.declare_dram_parameter("wq", [P, ncc, hd], BF16, isOutput=False)
    wk = nc.declare_dram_parameter("wk", [P, ncc, hd], BF16, isOutput=False)
    wv = nc.declare_dram_parameter("wv", [P, ncc, hd], BF16, isOutput=False)
    bq = nc.declare_dram_parameter("bq", [1, hd], BF16, isOutput=False)
    bk = nc.declare_dram_parameter("bk", [1, hd], BF16, isOutput=False)
    bv = nc.declare_dram_parameter("bv", [1, hd], BF16, isOutput=False)
    mbT = None
    if has_mask:
        mbT = nc.declare_dram_parameter("mbT", [t, f], BF16, isOutput=False)
    out_d = nc.declare_dram_parameter("out", [npair, d, f], F32, isOutput=True)

    with tile.TileContext(nc) as tc, ExitStack() as ctx:
        const = ctx.enter_context(tc.tile_pool(name="const", bufs=1))
        wpool = ctx.enter_context(tc.tile_pool(name="wpool", bufs=1))
        rqk = ctx.enter_context(tc.tile_pool(name="rqk", bufs=2 * npair))
        vpool = ctx.enter_context(tc.tile_pool(name="vpool", bufs=npair * nch))
        dpool = ctx.enter_context(tc.tile_pool(name="dpool", bufs=3, space="DRAM"))

        if has_bias:
            ones_row = const.tile([1, P], BF16)
            nc.vector.memset(ones_row[:], 1.0)
        ones_at_d = const.tile([d + 1, d], BF16)
        nc.vector.memset(ones_at_d[d:d + 1, :], 1.0)
        ident = const.tile([d + 1, d + 1], BF16)
        make_identity(nc, ident[:])

        w_s, b_s = {}, {}

        def load_weights():
            for name, wd, bd in (("q", wq, bq), ("k", wk, bk), ("v", wv, bv)):
                wt = wpool.tile([P, ncc, hd], BF16, tag=f"w{name}")
                for kc in range(ncc):
                    nc.sync.dma_start(out=wt[:, kc, :], in_=wd[:, kc, :])
                w_s[name] = wt
                if has_bias:
                    bt = wpool.tile([1, hd], BF16, tag=f"b{name}")
                    nc.sync.dma_start(out=bt[:], in_=bd[:])
                    b_s[name] = bt

        r_all = [{} for _ in range(npair)]
        vones_all = [[] for _ in range(npair)]
        cx_hold = {}
        fh = f // 2

        xpool = ctx.enter_context(tc.tile_pool(name="xpool", bufs=2))
        blkpool = ctx.enter_context(tc.tile_pool(name="blkpool", bufs=3))
        rv = ctx.enter_context(tc.tile_pool(name="rv", bufs=2))
        if exp_bufs is None:
            exp_bufs = 10 if has_mask else 12
        epool = ctx.enter_context(tc.tile_pool(name="epool", bufs=exp_bufs))
        opool = ctx.enter_context(tc.tile_pool(name="opool", bufs=2))
        spool = ctx.enter_context(tc.tile_pool(name="spool", bufs=1))
        mpool = None
        if has_mask:
            mpool = ctx.enter_context(tc.tile_pool(name="mpool", bufs=4))
        pp_mix = ctx.enter_context(tc.tile_pool(name="pp_mix", bufs=2, space="PSUM"))
        pp_ctx = ctx.enter_context(tc.tile_pool(name="pp_ctx", bufs=1, space="PSUM"))

        def emit_proj(j):
            xf_s = xpool.tile([P, ncc, bh], BF16, tag="xf")
            nc.sync.dma_start(out=xf_s[:], in_=xfT[j])
            xt_s = xpool.tile([P, ncc, bh], BF16, tag="xt")
            nc.sync.dma_start(out=xt_s[:], in_=xtT[j])
            yield
            for name, x_s in (("q", xf_s), ("k", xt_s), ("v", xt_s)):
                pj = pp_mix.tile([bh, hd], F32, tag="mix")
                if has_bias:
                    for ns, ne in _nsl(hd, NB):
                        nc.tensor.matmul(
                            pj[:, ns:ne], ones_row[:, :bh],
                            b_s[name][:, ns:ne],
                            start=True, stop=False,
                        )
                for kc in range(ncc):
                    first = kc == 0 and not has_bias
                    last = kc == ncc - 1
                    for ns, ne in _nsl(hd, NB):
                        nc.tensor.matmul(
                            pj[:, ns:ne], x_s[:, kc, :],
                            w_s[name][:, kc, ns:ne],
                            start=first, stop=last,
                        )
                    if kc % 3 == 2:
                        yield
                blk = blkpool.tile([bh, hd], BF16, tag="blk")
                if name == "k":
                    nc.vector.tensor_scalar_mul(blk[:], pj[:], float(ALPHA))
                else:
                    nc.vector.tensor_copy(blk[:], pj[:])
                dsc = dpool.tile([bh, hd], BF16, tag="dsc")
                nc.sync.dma_start(out=dsc[:], in_=blk[:])
                if name == "v":
                    r = rv.tile([d + 1, 2 * hd], BF16, tag="rv")
                else:
                    r = rqk.tile([d, 2 * hd], BF16, tag=f"r{name}")
                nc.sync.dma_start(
                    out=r[0:d, :],
                    in_=dsc[:].rearrange("(d two) n -> d (two n)", two=2),
                )
                r_all[j][name] = r
                yield
            r_v = r_all[j]["v"]
            nc.vector.memset(r_v[d:d + 1, :], 1.0)
            grp = 4
            for tg in range(0, nch, grp):
                gn = min(grp, nch - tg)
                vt_ps = pp_mix.tile([P, grp, d + 2], BF16, tag="mix")
                for ti in range(gn):
                    tcb = tg + ti
                    nc.tensor.transpose(
                        vt_ps[:, ti, 0:d + 1],
                        r_v[:, tcb * P:(tcb + 1) * P],
                        ident[:],
                    )
                    vo = vpool.tile([P, d + 1], BF16, tag="vones")
                    nc.vector.tensor_copy(vo[:], vt_ps[:, ti, 0:d + 1])
                    vones_all[j].append(vo)
                yield

        def emit_attn(j):
            r_q, r_k = r_all[j]["q"], r_all[j]["k"]
            ps_cx = pp_ctx.tile([d + 1, f], F32, tag="cx")
            for tcb in range(nch):
                exs = []
                for hf in range(2):
                    ps_sc = pp_mix.tile([P, fh], F32, tag="mix")
                    for ns, ne in _nsl(fh, NB):
                        nc.tensor.matmul(
                            ps_sc[:, ns:ne],
                            r_k[:, tcb * P:(tcb + 1) * P],
                            r_q[:, hf * fh + ns:hf * fh + ne],
                            start=True, stop=True,
                        )
                    if has_mask:
                        mt = mpool.tile([P, fh], BF16, tag="mb")
                        nc.sync.dma_start(
                            out=mt[:],
                            in_=mbT[tcb * P:(tcb + 1) * P, hf * fh:(hf + 1) * fh],
                        )
                        nc.vector.tensor_add(ps_sc[:], ps_sc[:], mt[:])
                    ex = epool.tile([P, fh], BF16, tag="exp")
                    nc.scalar.activation(
                        ex[:], ps_sc[:], mybir.ActivationFunctionType.Exp
                    )
                    exs.append(ex)
                REG = 512
                for hf in range(2):
                    for ns, ne in _nsl(fh, NB):
                        gs, ge = hf * fh + ns, hf * fh + ne
                        nc.tensor.matmul(
                            ps_cx[:, gs:ge],
                            vones_all[j][tcb][:],
                            exs[hf][:, ns:ne],
                            start=(tcb == 0 and gs % REG == 0),
                            stop=(tcb == nch - 1 and (ge % REG == 0 or ge == f)),
                        )
                yield
            cx_sb = opool.tile([d + 1, f], F32, tag="ctx")
            nc.vector.tensor_copy(cx_sb[:], ps_cx[:])
            cx_hold[j] = cx_sb
            yield

        def emit_norm(j):
            cx_sb = cx_hold[j]
            nc.vector.reciprocal(cx_sb[d:d + 1, :], cx_sb[d:d + 1, :])
            rc_bf = spool.tile([d + 1, f], BF16, tag="rcb")
            nc.vector.tensor_copy(rc_bf[d:d + 1, :], cx_sb[d:d + 1, :])
            yield
            bc_sb = spool.tile([d, f], F32, tag="bc")
            for hs, he in _nsl(f, min(fh, 1024)):
                ps_bc = pp_mix.tile([d, min(fh, 1024)], F32, tag="mix")
                for ns, ne in _nsl(he - hs, NB):
                    nc.tensor.matmul(
                        ps_bc[:, ns:ne], ones_at_d[d:d + 1, :],
                        rc_bf[d:d + 1, hs + ns:hs + ne],
                        start=True, stop=True,
                    )
                nc.vector.tensor_copy(bc_sb[:, hs:he], ps_bc[:, 0:he - hs])
                yield
            nc.vector.tensor_mul(cx_sb[0:d, :], cx_sb[0:d, :], bc_sb[:])
            nc.sync.dma_start(out=out_d[j], in_=cx_sb[0:d, :])
            yield

        pg0 = emit_proj(0)
        next(pg0)
        load_weights()
        for _ in pg0:
            pass
        ng = None
        for j in range(npair):
            pg = emit_proj(j + 1) if j + 1 < npair else None
            for _ in emit_attn(j):
                if pg is not None:
                    next(pg, None)
                if ng is not None:
                    next(ng, None)
            if pg is not None:
                for _ in pg:
                    pass
            if ng is not None:
                for _ in ng:
                    pass
            ng = emit_norm(j)
        for _ in ng:
            pass

    nc.finalize()
    return nc


_PROGRAM_CACHE = {}
TRACE = False
LAST_RESULTS = None


def _get_program(key):
    if key not in _PROGRAM_CACHE:
        if key == "fast":
            _PROGRAM_CACHE[key] = build_program_fast()
        else:
            has_mask, has_bias = key
            _PROGRAM_CACHE[key] = build_program_general(
                has_mask=has_mask, has_bias=has_bias
            )
    return _PROGRAM_CACHE[key]


def _split8(a):
    a8 = a.astype(NP_FP8)
    ar = (a - a8.astype(np.float32)).astype(NP_FP8)
    return a8, ar


def _kernel_fast(inputs, from_tensor, to_tensor):
    nc = _get_program("fast")
    bh = 2 * D

    def lay(a, inner):
        return np.ascontiguousarray(
            a.reshape(C // 128, 128, inner).transpose(1, 0, 2)
        )

    wmaps = {}
    for name, key in (("q", "Wq"), ("k", "Wk"), ("v", "Wv")):
        w8, wr = _split8(np.asarray(inputs[key], np.float32) * WS)
        wmaps[f"w8{name}"] = lay(w8, HD)
        wmaps[f"wr{name}"] = lay(wr, HD)

    def xprep(x, p):
        xb = np.ascontiguousarray(
            x[p // H, (p % H) * bh:(p % H + 1) * bh, :].T
        ).astype(np.float32)
        x8, xr = _split8(xb)
        return lay(x8, bh), lay(xr, bh)

    in_maps = []
    for core in range(N_CORES):
        pairs = [NPAIR * core + jj for jj in range(NPAIR)]
        xfm = np.empty((NPAIR, 2, 128, C // 128, bh), NP_FP8)
        xtm = np.empty_like(xfm)
        for jj, p in enumerate(pairs):
            xfm[jj, 0], xfm[jj, 1] = xprep(from_tensor, p)
            xtm[jj, 0], xtm[jj, 1] = xprep(to_tensor, p)
        m = {"xf": xfm, "xt": xtm}
        m.update(wmaps)
        in_maps.append(m)

    res = run_bass_kernel_spmd(
        nc, in_maps, core_ids=list(range(N_CORES)), trace=TRACE
    )
    global LAST_RESULTS
    LAST_RESULTS = res

    out = np.empty((B, HD, F), np.float32)
    nfc = F // 128
    for core in range(N_CORES):
        oc = res.results[core]["out_c"]   # (NPAIR, 128, nfc, D)
        os_ = res.results[core]["out_s"]  # (NPAIR, 128, nfc)
        for jj in range(NPAIR):
            p = NPAIR * core + jj
            b, h = p // H, p % H
            cx = oc[jj].astype(np.float32) / os_[jj][:, :, None]
            out[b, h * D:(h + 1) * D, :] = cx.transpose(2, 1, 0).reshape(D, F)
    return out


def _kernel_general(inputs, from_tensor, to_tensor, mb, has_mask, has_bias):
    nc = _get_program((has_mask, has_bias))
    bh = 2 * D

    def wprep(w):
        w = np.asarray(w, np.float32).astype(NP_BF16)
        return np.ascontiguousarray(
            w.reshape(C // 128, 128, HD).transpose(1, 0, 2)
        )

    wq = wprep(inputs["Wq"])
    wk = wprep(inputs["Wk"])
    wv = wprep(inputs["Wv"])
    bqv = np.asarray(inputs["bq"], np.float32).astype(NP_BF16).reshape(1, HD)
    bkv = np.asarray(inputs["bk"], np.float32).astype(NP_BF16).reshape(1, HD)
    bvv = np.asarray(inputs["bv"], np.float32).astype(NP_BF16).reshape(1, HD)

    def xprep(x, p):
        xb = x[p // H, (p % H) * bh:(p % H + 1) * bh, :].T.astype(NP_BF16)
        return np.ascontiguousarray(
            xb.reshape(C // 128, 128, bh).transpose(1, 0, 2)
        )

    in_maps = []
    for core in range(N_CORES):
        pairs = [NPAIR * core + jj for jj in range(NPAIR)]
        b = pairs[0] // H
        xf = np.stack([xprep(from_tensor, p) for p in pairs])
        xt = np.stack([xprep(to_tensor, p) for p in pairs])
        m = {
            "xfT": xf, "xtT": xt,
            "wq": wq, "wk": wk, "wv": wv,
            "bq": bqv, "bk": bkv, "bv": bvv,
        }
        if has_mask:
            m["mbT"] = np.ascontiguousarray(mb[b].T).astype(NP_BF16)
        in_maps.append(m)

    res = run_bass_kernel_spmd(
        nc, in_maps, core_ids=list(range(N_CORES)), trace=TRACE
    )
    global LAST_RESULTS
    LAST_RESULTS = res

    out = np.empty((B, HD, F), np.float32)
    for core in range(N_CORES):
        o = res.results[core]["out"]
        for jj in range(NPAIR):
            p = NPAIR * core + jj
            b, h = p // H, p % H
            out[b, h * D:(h + 1) * D, :] = o[jj]
    return out


def kernel(**inputs):
    from_tensor = np.asarray(inputs["from_tensor"], np.float32)
    to_tensor = np.asarray(inputs["to_tensor"], np.float32)
    mask = np.asarray(inputs["mask"], np.float32)

    mb = (1.0 - mask) * NEG  # (B, F, T) additive mask bias
    has_mask = bool(np.any(mb != 0.0))
    has_bias = bool(
        np.any(inputs["bq"]) or np.any(inputs["bk"]) or np.any(inputs["bv"])
    )
    if not has_mask and not has_bias:
        return _kernel_fast(inputs, from_tensor, to_tensor)
    return _kernel_general(
        inputs, from_tensor, to_tensor, mb, has_mask, has_bias
    )


# revision 22
# speedup vs baseline: 1.0091x; 1.0091x over previous
"""Trainium2 Bass kernel for nn_MultiHeaded_4080218931880.

Multi-headed attention with the reference's *raw reshape* head split:
    q = from @ Wq + bq                      # (B, F, HD)
    q_r = q.reshape(B, H, D, F)             # raw row-major reshape
    score = einsum('bhdf,bhdt->bhft', q_r, k_r) * alpha
    probs = softmax(score + (1-mask)*NEG, axis=-1)
    out = einsum('bhft,bhdt->bhdf', probs, v_r).reshape(B, H*D, F)

Because the reshape is raw, head h only touches rows [2*D*h, 2*D*(h+1))
of the (F, HD) projection output, so the 32 (b, h) pairs are fully
independent: 4 pairs per NeuronCore over 8 cores.

Fast path (mask all-ones, zero biases — the shipped problem instance):

  Projections via fp8 e4m3 DoubleRow, 3-term split (x8@W8 + xr@W8 +
  x8@Wr with W prescaled by WS=64 to stay in e4m3 normal range; the
  1/WS folds into the eviction scalar ops).  Terms are paired across
  adjacent c-chunks of the same kind, so each DoubleRow instruction
  contracts K_eff=256 at 0.5 cycles/row — 25% cheaper than bf16 and
  slightly MORE accurate (residual term cancels the x quantization).

  Score via fp8 DoubleRow 4-term split: q -> q8+qr, k -> k8+kr (DVE
  evictions straight from the projection PSUM), stacked [q8;qr] on
  partitions and [k8;k8],[kr;kr] on the two DoubleRow j-subtiles.  One
  instruction per 512-slice computes exact-to-~fp16^2 scores at HALF
  the bf16 PE cost.  alpha is applied for free via the exp
  activation's scale parameter.

  ctx stays bf16 (exp tiles bf16, v bf16 with a ones row so the ctx
  accumulator's row D carries the softmax denominator).  The per-pair
  (D+1, F) accumulator is DMA'd out raw; the final divide by the
  denominator row happens on host during the gather/unshard step.

Fallback path (general mask/bias) keeps the previous all-bf16 program.
"""

import numpy as np
from contextlib import ExitStack

import concourse.bass as bass
import concourse.bacc as bacc
import concourse.tile as tile
from concourse import mybir
from concourse.bass_utils import run_bass_kernel_spmd
from concourse.masks import make_identity

BF16 = mybir.dt.bfloat16
F32 = mybir.dt.float32
FP8 = mybir.dt.float8e4
NP_BF16 = mybir.dt.np(mybir.dt.bfloat16)
NP_FP8 = mybir.dt.np(mybir.dt.float8e4)

# Problem dims (hardcoded; harness runs kernel.py standalone).
B, F, T, C = 2, 2048, 2048, 1024
H, D = 16, 64
HD = H * D
ALPHA = 1.0 / np.sqrt(np.float32(D)).astype(np.float32)
NEG = -100000.0
N_CORES = 8
NPAIR = (B * H) // N_CORES  # 4 (b,h) pairs per core
P = 128
DR = mybir.MatmulPerfMode.DoubleRow
WS = 64.0  # fp8 weight prescale (power of two; folded back at eviction)

REAL_DIMS = dict(npair=NPAIR, c=C, hd=HD, d=D, f=F, t=T)


def _nsl(total, step):
    """Split [0, total) into <=step slices (matmul moving free-dim limit)."""
    return [(s, min(s + step, total)) for s in range(0, total, step)]


def build_program_fast(dims=None, alpha=float(ALPHA)):
    dims = dims or REAL_DIMS
    npair, c, hd, d, f, t = (
        dims["npair"], dims["c"], dims["hd"], dims["d"], dims["f"], dims["t"],
    )
    bh = 2 * d
    ncc = c // P
    ncp = ncc // 2
    nch = t // P       # t'-chunks (score partition dim / ctx contraction)
    nfc = f // P       # f'-chunks (transposed-ctx partition dim)
    NB = 512
    fh = f // 2
    che = hd // P      # score chunks per e-half of the ks layout
    cpf = fh // P      # f'-chunks per exp half
    mult = mybir.AluOpType.mult
    subtract = mybir.AluOpType.subtract

    nc = bacc.Bacc(None, target_bir_lowering=False, debug=True)
    # dim1 of x params: 0 = fp8 value, 1 = fp8 residual
    xf = nc.declare_dram_parameter("xf", [npair, 2, P, ncc, bh], FP8, isOutput=False)
    xt = nc.declare_dram_parameter("xt", [npair, 2, P, ncc, bh], FP8, isOutput=False)
    wparams = {}
    for name in ("q", "k", "v"):
        wparams[name] = (
            nc.declare_dram_parameter(f"w8{name}", [P, ncc, hd], FP8, isOutput=False),
            nc.declare_dram_parameter(f"wr{name}", [P, ncc, hd], FP8, isOutput=False),
        )
    # transposed-ctx outputs: out_c[j][f', fc, dd] = ctx[dd, fc*128+f'],
    # out_s[j][f', fc] = softmax denominator for column fc*128+f'
    out_c = nc.declare_dram_parameter("out_c", [npair, P, nfc, d], BF16, isOutput=True)
    out_s = nc.declare_dram_parameter("out_s", [npair, P, nfc], F32, isOutput=True)

    with tile.TileContext(nc) as tc, ExitStack() as ctx:
        const = ctx.enter_context(tc.tile_pool(name="const", bufs=1))
        wpool = ctx.enter_context(tc.tile_pool(name="wpool", bufs=1))
        rqk = ctx.enter_context(tc.tile_pool(name="rqk", bufs=4))
        vpool = ctx.enter_context(tc.tile_pool(name="vpool", bufs=npair * nch))

        ident = const.tile([d + 1, d + 1], BF16)
        make_identity(nc, ident[:])

        w8_s, wr_s = {}, {}

        def load_weights(names):
            # one DMA per tensor-half: HWDGE queue slots are the scarce
            # resource (625ns serial each), not bandwidth.  v is deferred
            # so pair-0's eviction DMAs don't queue behind it.
            for name in names:
                w8d, wrd = wparams[name]
                w8t_ = wpool.tile([P, ncc, hd], FP8, tag=f"w8{name}")
                nc.sync.dma_start(out=w8t_[:], in_=w8d[:])
                wrt_ = wpool.tile([P, ncc, hd], FP8, tag=f"wr{name}")
                nc.sync.dma_start(out=wrt_[:], in_=wrd[:])
                w8_s[name] = w8t_
                wr_s[name] = wrt_

        r_all = [{} for _ in range(npair)]
        vones_all = [[] for _ in range(npair)]

        xpool = ctx.enter_context(tc.tile_pool(name="xpool", bufs=2))
        blkpool = ctx.enter_context(tc.tile_pool(name="blkpool", bufs=6))
        rv = ctx.enter_context(tc.tile_pool(name="rv", bufs=2))
        epool = ctx.enter_context(tc.tile_pool(name="epool", bufs=16))
        opool = ctx.enter_context(tc.tile_pool(name="opool", bufs=4))
        # PSUM map (all 128 partitions, 8 banks of 2KB):
        #   banks 0-3: two score slots (128, fh<=1024) f32 -- exp ping-pong
        #   banks 4-5: transposed-ctx accumulator (128, nfc, d) f32,
        #              256B chunks, accumulated WITHOUT start (memset once
        #              per pair) so sub-bank chunks never zero each other
        #   bank 6:    projection accumulator (128, 512) f32; also hosts
        #              the v-transpose staging tile between projections
        #   bank 7:    denominator accumulator (128, nfc) f32
        pp_score = ctx.enter_context(tc.tile_pool(name="pp_score", bufs=2, space="PSUM"))
        pp_ctxT = ctx.enter_context(tc.tile_pool(name="pp_ctxT", bufs=1, space="PSUM"))
        pp_pj = ctx.enter_context(tc.tile_pool(name="pp_pj", bufs=1, space="PSUM"))
        pp_s = ctx.enter_context(tc.tile_pool(name="pp_s", bufs=1, space="PSUM"))

        def proj_mm(x_s, name, pj, ns, ne, gi, full=False):
            """One term-group of DoubleRow projection matmuls for a slice."""
            terms = ((0, w8_s[name]), (1, w8_s[name]), (0, wr_s[name]))
            kind, ws = terms[gi]
            out = pj[:, ns:ne] if full else pj[:, 0:ne - ns]
            for cp in range(ncp):
                nc.tensor.matmul(
                    out,
                    x_s[:, kind, 2 * cp:2 * cp + 2, :],
                    ws[:, 2 * cp:2 * cp + 2, ns:ne],
                    start=(gi == 0 and cp == 0),
                    stop=(gi == 2 and cp == ncp - 1),
                    perf_mode=DR,
                )

        def evict_qk_slice(blk, hqr, ns, ne):
            nc.vector.tensor_scalar_mul(hqr[:, 0, ns:ne], blk[:], 1.0 / WS)
            nc.vector.scalar_tensor_tensor(
                hqr[:, 1, ns:ne], blk[:], 1.0 / WS, hqr[:, 0, ns:ne],
                op0=mult, op1=subtract,
            )

        def bounce_qk(j, name, hqr):
            """Direct SBUF->SBUF reshape DMAs (flat element-stream order
            realizes the raw (2d, hd) -> (d, 2*hd) reshape)."""
            if name == "q":
                # moving side: flat [q8;qr] stack; the DoubleRow
                # j-duplication happens via a stride-0 AP
                s = rqk.tile([P, f], FP8, tag="qs")
                nc.sync.dma_start(out=s[0:d, :], in_=hqr[:, 0, :])
                nc.sync.dma_start(out=s[d:2 * d, :], in_=hqr[:, 1, :])
            else:
                # stationary side, e-major (P, e, j, c): j=0 carries k8,
                # j=1 kr; both partition-halves hold the same data
                s = rqk.tile([P, 2, 2, hd], FP8, tag="ks")
                nc.sync.dma_start(out=s[0:d], in_=hqr[:])
                nc.sync.dma_start(out=s[d:2 * d], in_=hqr[:])
            r_all[j][name] = s

        def emit_transposes(j):
            r_v = r_all[j]["v"]
            nc.vector.memset(r_v[d:d + 1, :], 1.0)
            grp = 2
            for tg in range(0, nch, grp):
                gn = min(grp, nch - tg)
                vt_ps = pp_pj.tile([P, grp, d + 2], BF16, tag="pj")
                for ti in range(gn):
                    tcb = tg + ti
                    nc.tensor.transpose(
                        vt_ps[:, ti, 0:d + 1],
                        r_v[:, tcb * P:(tcb + 1) * P],
                        ident[:],
                    )
                    vo = vpool.tile([P, d + 1], BF16, tag="vones")
                    nc.vector.tensor_copy(vo[:], vt_ps[:, ti, 0:d + 1])
                    vones_all[j].append(vo)
                yield

        QKDONE = "qkdone"

        def emit_proj(j, gate_x=False):
            xf_s = xpool.tile([P, 2, ncc, bh], FP8, tag="xf")
            xt_s = xpool.tile([P, 2, ncc, bh], FP8, tag="xt")
            eng = nc.gpsimd if gate_x else nc.sync
            eng.dma_start(
                out=xf_s[:], in_=xf[j].rearrange("k p c b -> p k c b")
            )
            eng.dma_start(
                out=xt_s[:], in_=xt[j].rearrange("k p c b -> p k c b")
            )
            yield
            for name, x_s in (("q", xf_s), ("k", xt_s), ("v", xt_s)):
                if name == "v":
                    hv = blkpool.tile([bh, hd], BF16, tag="hv")
                else:
                    hqr = blkpool.tile([bh, 2, hd], FP8, tag="hqr")
                for ns, ne in _nsl(hd, NB):
                    pj = pp_pj.tile([bh, min(NB, hd)], F32, tag="pj")
                    for gi in range(3):
                        proj_mm(x_s, name, pj, ns, ne, gi)
                        yield
                    # evictions fold the 1/WS weight prescale back in; the
                    # per-slice copy frees the proj PSUM bank quickly
                    if name == "v":
                        nc.vector.tensor_scalar_mul(
                            hv[:, ns:ne], pj[:, 0:ne - ns], 1.0 / WS)
                    else:
                        blk = blkpool.tile([bh, min(NB, hd)], F32, tag="blk")
                        nc.vector.tensor_copy(blk[:], pj[:, 0:ne - ns])
                        evict_qk_slice(blk[:, 0:ne - ns], hqr, ns, ne)
                    yield
                if name == "v":
                    r = rv.tile([d + 1, 2 * hd], BF16, tag="rv")
                    nc.sync.dma_start(out=r[0:d, :], in_=hv[:])
                    r_all[j]["v"] = r
                else:
                    bounce_qk(j, name, hqr)
                yield
            yield from emit_transposes(j)

        def emit_proj0():
            """Pair-0 startup: q/k projections accumulate full-width in the
            (still idle) score banks so they track their weight DMAs in
            parallel; v weights are data-gated behind the ks bounce."""
            xf_s = xpool.tile([P, 2, ncc, bh], FP8, tag="xf")
            nc.sync.dma_start(
                out=xf_s[:], in_=xf[0].rearrange("k p c b -> p k c b")
            )
            xt_s = xpool.tile([P, 2, ncc, bh], FP8, tag="xt")
            nc.sync.dma_start(
                out=xt_s[:], in_=xt[0].rearrange("k p c b -> p k c b")
            )
            yield
            pj_q = pp_score.tile([bh, hd], F32, tag="sc")
            pj_k = pp_score.tile([bh, hd], F32, tag="sc")
            for gi in range(2):
                for ns, ne in _nsl(hd, NB):
                    proj_mm(xf_s, "q", pj_q, ns, ne, gi, full=True)
                for ns, ne in _nsl(hd, NB):
                    proj_mm(xt_s, "k", pj_k, ns, ne, gi, full=True)
            for pj, name in ((pj_q, "q"), (pj_k, "k")):
                for ns, ne in _nsl(hd, NB):
                    proj_mm(xf_s if name == "q" else xt_s,
                            name, pj, ns, ne, 2, full=True)
                hqr = blkpool.tile([bh, 2, hd], FP8, tag="hqr")
                blk = blkpool.tile([bh, hd], F32, tag="blk0")
                nc.vector.tensor_copy(blk[:], pj[:])
                evict_qk_slice(blk[:], hqr, 0, hd)
                bounce_qk(0, name, hqr)
            # v weights issue from the (idle) Pool engine behind a dummy
            # read of the ks tile, so they reach the shared DMA engines only
            # after the critical ks bounce DMAs have executed
            gate = blkpool.tile([1, 1], FP8, tag="gate")
            nc.gpsimd.tensor_copy(gate[:], r_all[0]["k"][0:1, 0:1, 0:1, 0:1])
            yield QKDONE
            for name in ("v",):
                w8d, wrd = wparams[name]
                w8t_ = wpool.tile([P, ncc, hd], FP8, tag=f"w8{name}")
                wrt_ = wpool.tile([P, ncc, hd], FP8, tag=f"wr{name}")
                nc.gpsimd.dma_start(out=w8t_[:], in_=w8d[:])
                nc.gpsimd.dma_start(out=wrt_[:], in_=wrd[:])
                w8_s[name] = w8t_
                wr_s[name] = wrt_
            yield
            hv = blkpool.tile([bh, hd], BF16, tag="hv")
            for ns, ne in _nsl(hd, NB):
                pj = pp_pj.tile([bh, min(NB, hd)], F32, tag="pj")
                for gi in range(3):
                    proj_mm(xt_s, "v", pj, ns, ne, gi)
                    yield
                nc.vector.tensor_scalar_mul(
                    hv[:, ns:ne], pj[:, 0:ne - ns], 1.0 / WS)
                yield
            r = rv.tile([d + 1, 2 * hd], BF16, tag="rv")
            nc.sync.dma_start(out=r[0:d, :], in_=hv[:])
            r_all[0]["v"] = r
            yield
            yield from emit_transposes(0)

        def make_ctx_chunk(j, st):
            def ctx_chunk(tcb):
                vo = vones_all[j][tcb]
                last = tcb == nch - 1
                for fc in range(nfc):
                    ex = st["exs"][tcb][fc // cpf]
                    exsl = ex[:, (fc % cpf) * P:(fc % cpf) * P + P]
                    nc.tensor.matmul(
                        st["ps_cx"][:, fc, :], exsl, vo[:, 0:d],
                        start=False, stop=last, skip_group_check=True,
                    )
                    nc.tensor.matmul(
                        st["ps_s"][:, fc:fc + 1], exsl, vo[:, d:d + 1],
                        start=False, stop=last, skip_group_check=True,
                    )
            return ctx_chunk

        def emit_attn_chunks(j, st):
            qs, ks = r_all[j]["q"], r_all[j]["k"]
            ps_cx = pp_ctxT.tile([P, nfc, d], F32, tag="cx")
            ps_s = pp_s.tile([P, nfc], F32, tag="s")
            # accumulators run WITHOUT start flags (sub-bank chunks would
            # zero each other's region): zero them explicitly instead
            nc.vector.memset(ps_cx[:], 0.0)
            nc.vector.memset(ps_s[:], 0.0)
            st["ps_cx"] = ps_cx
            st["ps_s"] = ps_s
            st["exs"] = {}
            st["ctx_i"] = 0
            ctx_chunk = make_ctx_chunk(j, st)
            for tcb in range(nch):
                exs = []
                for hf in range(2):
                    ps_sc = pp_score.tile([P, fh], F32, tag="sc")
                    for ns, ne in _nsl(fh, NB):
                        nc.tensor.matmul(
                            ps_sc[:, ns:ne],
                            ks[:, tcb // che, :,
                               (tcb % che) * P:(tcb % che) * P + P],
                            qs[:, hf * fh + ns:hf * fh + ne]
                                .unsqueeze(1).broadcast_to((P, 2, ne - ns)),
                            start=True, stop=True,
                            perf_mode=DR,
                        )
                    ex = epool.tile([P, fh], BF16, tag="exp")
                    nc.scalar.activation(
                        ex[:], ps_sc[:], mybir.ActivationFunctionType.Exp,
                        scale=alpha,
                    )
                    exs.append(ex)
                st["exs"][tcb] = exs
                # ctx lags >=1 chunk (so PE never waits on the current exp)
                # and is additionally gated on the v transposes having been
                # emitted (pair 0: v projection overlaps early attention)
                while st["ctx_i"] < tcb and st["ctx_i"] < len(vones_all[j]):
                    ctx_chunk(st["ctx_i"])
                    st["ctx_i"] += 1
                yield

        def emit_attn_tail(j, st):
            ctx_chunk = make_ctx_chunk(j, st)
            while st["ctx_i"] < nch:
                ctx_chunk(st["ctx_i"])
                st["ctx_i"] += 1
            # bf16 output, evicted in halves so the first DMA overlaps the
            # second copy
            cx_sb = opool.tile([P, nfc, d], BF16, tag="ctx")
            hn = nfc // 2
            nc.vector.tensor_copy(cx_sb[:, 0:hn, :], st["ps_cx"][:, 0:hn, :])
            nc.sync.dma_start(out=out_c[j, :, 0:hn, :], in_=cx_sb[:, 0:hn, :])
            nc.vector.tensor_copy(cx_sb[:, hn:nfc, :], st["ps_cx"][:, hn:nfc, :])
            s_sb = opool.tile([P, nfc], F32, tag="s")
            nc.vector.tensor_copy(s_sb[:], st["ps_s"][:])
            nc.sync.dma_start(out=out_c[j, :, hn:nfc, :], in_=cx_sb[:, hn:nfc, :])
            nc.sync.dma_start(out=out_s[j], in_=s_sb[:])

        # software pipeline: a FIFO of projection generators pumped a few
        # steps per attention chunk, so projection/transpose work spreads
        # into the PE slack between score and ctx matmuls and never bunches
        # at pair boundaries.
        pending = []

        def pump(n):
            for _ in range(n):
                while pending:
                    try:
                        next(pending[0])
                        break
                    except StopIteration:
                        pending.pop(0)
                else:
                    return

        pg0 = emit_proj0()
        next(pg0)        # pair-0 x loads issue before the weight DMAs
        load_weights(("q", "k"))
        for step in pg0:
            if step == QKDONE:
                break
        pending.append(pg0)
        pgs = {0: pg0}
        for j in range(npair):
            if j + 1 < npair:
                g = emit_proj(j + 1, gate_x=(j == 0))
                pgs[j + 1] = g
                pending.append(g)
            # pair j's q/k score tiles must be emitted before its attention
            while "k" not in r_all[j] or "q" not in r_all[j]:
                pump(1)
            st = {}
            for _ in emit_attn_chunks(j, st):
                pump(3)
            # pair j's transposes must all be emitted before the ctx tail
            gj = pgs.get(j)
            if gj is not None:
                for _ in gj:
                    pass
                if gj in pending:
                    pending.remove(gj)
            emit_attn_tail(j, st)
        for g in pending:
            for _ in g:
                pass

    nc.finalize()
    return nc


def build_program_general(has_mask=False, has_bias=True, dims=None, exp_bufs=None):
    """All-bf16 fallback program (handles mask and bias)."""
    dm = dims or REAL_DIMS
    npair, c, hd, d, f, t = (
        dm["npair"], dm["c"], dm["hd"], dm["d"], dm["f"], dm["t"],
    )
    bh = 2 * d          # row-block height of x per (b,h) pair
    ncc = c // P        # contraction chunks for projections
    nch = t // P        # t' chunks for attention
    NB = 512            # matmul PSUM-write limit: one 2KB bank (512 f32)

    nc = bacc.Bacc(None, target_bir_lowering=False, debug=True)
    xfT = nc.declare_dram_parameter("xfT", [npair, P, ncc, bh], BF16, isOutput=False)
    xtT = nc.declare_dram_parameter("xtT", [npair, P, ncc, bh], BF16, isOutput=False)
    wq = nc.declare_dram_parameter("wq", [P, ncc, hd], BF16, isOutput=False)
    wk = nc.declare_dram_parameter("wk", [P, ncc, hd], BF16, isOutput=False)
    wv = nc.declare_dram_parameter("wv", [P, ncc, hd], BF16, isOutput=False)
    bq = nc.declare_dram_parameter("bq", [1, hd], BF16, isOutput=False)
    bk = nc.declare_dram_parameter("bk", [1, hd], BF16, isOutput=False)
    bv = nc.declare_dram_parameter("bv", [1, hd], BF16, isOutput=False)
    mbT = None
    if has_mask:
        mbT = nc.declare_dram_parameter("mbT", [t, f], BF16, isOutput=False)
    out_d = nc.declare_dram_parameter("out", [npair, d, f], F32, isOutput=True)

    with tile.TileContext(nc) as tc, ExitStack() as ctx:
        const = ctx.enter_context(tc.tile_pool(name="const", bufs=1))
        wpool = ctx.enter_context(tc.tile_pool(name="wpool", bufs=1))
        rqk = ctx.enter_context(tc.tile_pool(name="rqk", bufs=2 * npair))
        vpool = ctx.enter_context(tc.tile_pool(name="vpool", bufs=npair * nch))
        dpool = ctx.enter_context(tc.tile_pool(name="dpool", bufs=3, space="DRAM"))

        if has_bias:
            ones_row = const.tile([1, P], BF16)
            nc.vector.memset(ones_row[:], 1.0)
        ones_at_d = const.tile([d + 1, d], BF16)
        nc.vector.memset(ones_at_d[d:d + 1, :], 1.0)
        ident = const.tile([d + 1, d + 1], BF16)
        make_identity(nc, ident[:])

        w_s, b_s = {}, {}

        def load_weights():
            for name, wd, bd in (("q", wq, bq), ("k", wk, bk), ("v", wv, bv)):
                wt = wpool.tile([P, ncc, hd], BF16, tag=f"w{name}")
                for kc in range(ncc):
                    nc.sync.dma_start(out=wt[:, kc, :], in_=wd[:, kc, :])
                w_s[name] = wt
                if has_bias:
                    bt = wpool.tile([1, hd], BF16, tag=f"b{name}")
                    nc.sync.dma_start(out=bt[:], in_=bd[:])
                    b_s[name] = bt

        r_all = [{} for _ in range(npair)]
        vones_all = [[] for _ in range(npair)]
        cx_hold = {}
        fh = f // 2

        xpool = ctx.enter_context(tc.tile_pool(name="xpool", bufs=2))
        blkpool = ctx.enter_context(tc.tile_pool(name="blkpool", bufs=3))
        rv = ctx.enter_context(tc.tile_pool(name="rv", bufs=2))
        if exp_bufs is None:
            exp_bufs = 10 if has_mask else 12
        epool = ctx.enter_context(tc.tile_pool(name="epool", bufs=exp_bufs))
        opool = ctx.enter_context(tc.tile_pool(name="opool", bufs=2))
        spool = ctx.enter_context(tc.tile_pool(name="spool", bufs=1))
        mpool = None
        if has_mask:
            mpool = ctx.enter_context(tc.tile_pool(name="mpool", bufs=4))
        pp_mix = ctx.enter_context(tc.tile_pool(name="pp_mix", bufs=2, space="PSUM"))
        pp_ctx = ctx.enter_context(tc.tile_pool(name="pp_ctx", bufs=1, space="PSUM"))

        def emit_proj(j):
            xf_s = xpool.tile([P, ncc, bh], BF16, tag="xf")
            nc.sync.dma_start(out=xf_s[:], in_=xfT[j])
            xt_s = xpool.tile([P, ncc, bh], BF16, tag="xt")
            nc.sync.dma_start(out=xt_s[:], in_=xtT[j])
            yield
            for name, x_s in (("q", xf_s), ("k", xt_s), ("v", xt_s)):
                pj = pp_mix.tile([bh, hd], F32, tag="mix")
                if has_bias:
                    for ns, ne in _nsl(hd, NB):
                        nc.tensor.matmul(
                            pj[:, ns:ne], ones_row[:, :bh],
                            b_s[name][:, ns:ne],
                            start=True, stop=False,
                        )
                for kc in range(ncc):
                    first = kc == 0 and not has_bias
                    last = kc == ncc - 1
                    for ns, ne in _nsl(hd, NB):
                        nc.tensor.matmul(
                            pj[:, ns:ne], x_s[:, kc, :],
                            w_s[name][:, kc, ns:ne],
                            start=first, stop=last,
                        )
                    if kc % 3 == 2:
                        yield
                blk = blkpool.tile([bh, hd], BF16, tag="blk")
                if name == "k":
                    nc.vector.tensor_scalar_mul(blk[:], pj[:], float(ALPHA))
                else:
                    nc.vector.tensor_copy(blk[:], pj[:])
                dsc = dpool.tile([bh, hd], BF16, tag="dsc")
                nc.sync.dma_start(out=dsc[:], in_=blk[:])
                if name == "v":
                    r = rv.tile([d + 1, 2 * hd], BF16, tag="rv")
                else:
                    r = rqk.tile([d, 2 * hd], BF16, tag=f"r{name}")
                nc.sync.dma_start(
                    out=r[0:d, :],
                    in_=dsc[:].rearrange("(d two) n -> d (two n)", two=2),
                )
                r_all[j][name] = r
                yield
            r_v = r_all[j]["v"]
            nc.vector.memset(r_v[d:d + 1, :], 1.0)
            grp = 4
            for tg in range(0, nch, grp):
                gn = min(grp, nch - tg)
                vt_ps = pp_mix.tile([P, grp, d + 2], BF16, tag="mix")
                for ti in range(gn):
                    tcb = tg + ti
                    nc.tensor.transpose(
                        vt_ps[:, ti, 0:d + 1],
                        r_v[:, tcb * P:(tcb + 1) * P],
                        ident[:],
                    )
                    vo = vpool.tile([P, d + 1], BF16, tag="vones")
                    nc.vector.tensor_copy(vo[:], vt_ps[:, ti, 0:d + 1])
                    vones_all[j].append(vo)
                yield

        def emit_attn(j):
            r_q, r_k = r_all[j]["q"], r_all[j]["k"]
            ps_cx = pp_ctx.tile([d + 1, f], F32, tag="cx")
            for tcb in range(nch):
                exs = []
                for hf in range(2):
                    ps_sc = pp_mix.tile([P, fh], F32, tag="mix")
                    for ns, ne in _nsl(fh, NB):
                        nc.tensor.matmul(
                            ps_sc[:, ns:ne],
                            r_k[:, tcb * P:(tcb + 1) * P],
                            r_q[:, hf * fh + ns:hf * fh + ne],
                            start=True, stop=True,
                        )
                    if has_mask:
                        mt = mpool.tile([P, fh], BF16, tag="mb")
                        nc.sync.dma_start(
                            out=mt[:],
                            in_=mbT[tcb * P:(tcb + 1) * P, hf * fh:(hf + 1) * fh],
                        )
                        nc.vector.tensor_add(ps_sc[:], ps_sc[:], mt[:])
                    ex = epool.tile([P, fh], BF16, tag="exp")
                    nc.scalar.activation(
                        ex[:], ps_sc[:], mybir.ActivationFunctionType.Exp
                    )
                    exs.append(ex)
                REG = 512
                for hf in range(2):
                    for ns, ne in _nsl(fh, NB):
                        gs, ge = hf * fh + ns, hf * fh + ne
                        nc.tensor.matmul(
                            ps_cx[:, gs:ge],
                            vones_all[j][tcb][:],
                            exs[hf][:, ns:ne],
                            start=(tcb == 0 and gs % REG == 0),
                            stop=(tcb == nch - 1 and (ge % REG == 0 or ge == f)),
                        )
                yield
            cx_sb = opool.tile([d + 1, f], F32, tag="ctx")
            nc.vector.tensor_copy(cx_sb[:], ps_cx[:])
            cx_hold[j] = cx_sb
            yield

        def emit_norm(j):
            cx_sb = cx_hold[j]
            nc.vector.reciprocal(cx_sb[d:d + 1, :], cx_sb[d:d + 1, :])
            rc_bf = spool.tile([d + 1, f], BF16, tag="rcb")
            nc.vector.tensor_copy(rc_bf[d:d + 1, :], cx_sb[d:d + 1, :])
            yield
            bc_sb = spool.tile([d, f], F32, tag="bc")
            for hs, he in _nsl(f, min(fh, 1024)):
                ps_bc = pp_mix.tile([d, min(fh, 1024)], F32, tag="mix")
                for ns, ne in _nsl(he - hs, NB):
                    nc.tensor.matmul(
                        ps_bc[:, ns:ne], ones_at_d[d:d + 1, :],
                        rc_bf[d:d + 1, hs + ns:hs + ne],
                        start=True, stop=True,
                    )
                nc.vector.tensor_copy(bc_sb[:, hs:he], ps_bc[:, 0:he - hs])
                yield
            nc.vector.tensor_mul(cx_sb[0:d, :], cx_sb[0:d, :], bc_sb[:])
            nc.sync.dma_start(out=out_d[j], in_=cx_sb[0:d, :])
            yield

        pg0 = emit_proj(0)
        next(pg0)
        load_weights()
        for _ in pg0:
            pass
        ng = None
        for j in range(npair):
            pg = emit_proj(j + 1) if j + 1 < npair else None
            for _ in emit_attn(j):
                if pg is not None:
                    next(pg, None)
                if ng is not None:
                    next(ng, None)
            if pg is not None:
                for _ in pg:
                    pass
            if ng is not None:
                for _ in ng:
                    pass
            ng = emit_norm(j)
        for _ in ng:
            pass

    nc.finalize()
    return nc


_PROGRAM_CACHE = {}
TRACE = False
LAST_RESULTS = None


def _get_program(key):
    if key not in _PROGRAM_CACHE:
        if key == "fast":
            _PROGRAM_CACHE[key] = build_program_fast()
        else:
            has_mask, has_bias = key
            _PROGRAM_CACHE[key] = build_program_general(
                has_mask=has_mask, has_bias=has_bias
            )
    return _PROGRAM_CACHE[key]


def _split8(a):
    a8 = a.astype(NP_FP8)
    ar = (a - a8.astype(np.float32)).astype(NP_FP8)
    return a8, ar


def _kernel_fast(inputs, from_tensor, to_tensor):
    nc = _get_program("fast")
    bh = 2 * D

    def lay(a, inner):
        return np.ascontiguousarray(
            a.reshape(C // 128, 128, inner).transpose(1, 0, 2)
        )

    wmaps = {}
    for name, key in (("q", "Wq"), ("k", "Wk"), ("v", "Wv")):
        w8, wr = _split8(np.asarray(inputs[key], np.float32) * WS)
        wmaps[f"w8{name}"] = lay(w8, HD)
        wmaps[f"wr{name}"] = lay(wr, HD)

    def xprep(x, p):
        xb = np.ascontiguousarray(
            x[p // H, (p % H) * bh:(p % H + 1) * bh, :].T
        ).astype(np.float32)
        x8, xr = _split8(xb)
        return lay(x8, bh), lay(xr, bh)

    in_maps = []
    for core in range(N_CORES):
        pairs = [NPAIR * core + jj for jj in range(NPAIR)]
        xfm = np.empty((NPAIR, 2, 128, C // 128, bh), NP_FP8)
        xtm = np.empty_like(xfm)
        for jj, p in enumerate(pairs):
            xfm[jj, 0], xfm[jj, 1] = xprep(from_tensor, p)
            xtm[jj, 0], xtm[jj, 1] = xprep(to_tensor, p)
        m = {"xf": xfm, "xt": xtm}
        m.update(wmaps)
        in_maps.append(m)

    res = run_bass_kernel_spmd(
        nc, in_maps, core_ids=list(range(N_CORES)), trace=TRACE
    )
    global LAST_RESULTS
    LAST_RESULTS = res

    out = np.empty((B, HD, F), np.float32)
    nfc = F // 128
    for core in range(N_CORES):
        oc = res.results[core]["out_c"]   # (NPAIR, 128, nfc, D)
        os_ = res.results[core]["out_s"]  # (NPAIR, 128, nfc)
        for jj in range(NPAIR):
            p = NPAIR * core + jj
            b, h = p // H, p % H
            cx = oc[jj].astype(np.float32) / os_[jj][:, :, None]
            out[b, h * D:(h + 1) * D, :] = cx.transpose(2, 1, 0).reshape(D, F)
    return out


def _kernel_general(inputs, from_tensor, to_tensor, mb, has_mask, has_bias):
    nc = _get_program((has_mask, has_bias))
    bh = 2 * D

    def wprep(w):
        w = np.asarray(w, np.float32).astype(NP_BF16)
        return np.ascontiguousarray(
            w.reshape(C // 128, 128, HD).transpose(1, 0, 2)
        )

    wq = wprep(inputs["Wq"])
    wk = wprep(inputs["Wk"])
    wv = wprep(inputs["Wv"])
    bqv = np.asarray(inputs["bq"], np.float32).astype(NP_BF16).reshape(1, HD)
    bkv = np.asarray(inputs["bk"], np.float32).astype(NP_BF16).reshape(1, HD)
    bvv = np.asarray(inputs["bv"], np.float32).astype(NP_BF16).reshape(1, HD)

    def xprep(x, p):
        xb = x[p // H, (p % H) * bh:(p % H + 1) * bh, :].T.astype(NP_BF16)
        return np.ascontiguousarray(
            xb.reshape(C // 128, 128, bh).transpose(1, 0, 2)
        )

    in_maps = []
    for core in range(N_CORES):
        pairs = [NPAIR * core + jj for jj in range(NPAIR)]
        b = pairs[0] // H
        xf = np.stack([xprep(from_tensor, p) for p in pairs])
        xt = np.stack([xprep(to_tensor, p) for p in pairs])
        m = {
            "xfT": xf, "xtT": xt,
            "wq": wq, "wk": wk, "wv": wv,
            "bq": bqv, "bk": bkv, "bv": bvv,
        }
        if has_mask:
            m["mbT"] = np.ascontiguousarray(mb[b].T).astype(NP_BF16)
        in_maps.append(m)

    res = run_bass_kernel_spmd(
        nc, in_maps, core_ids=list(range(N_CORES)), trace=TRACE
    )
    global LAST_RESULTS
    LAST_RESULTS = res

    out = np.empty((B, HD, F), np.float32)
    for core in range(N_CORES):
        o = res.results[core]["out"]
        for jj in range(NPAIR):
            p = NPAIR * core + jj
            b, h = p // H, p % H
            out[b, h * D:(h + 1) * D, :] = o[jj]
    return out


def kernel(**inputs):
    from_tensor = np.asarray(inputs["from_tensor"], np.float32)
    to_tensor = np.asarray(inputs["to_tensor"], np.float32)
    mask = np.asarray(inputs["mask"], np.float32)

    mb = (1.0 - mask) * NEG  # (B, F, T) additive mask bias
    has_mask = bool(np.any(mb != 0.0))
    has_bias = bool(
        np.any(inputs["bq"]) or np.any(inputs["bk"]) or np.any(inputs["bv"])
    )
    if not has_mask and not has_bias:
        return _kernel_fast(inputs, from_tensor, to_tensor)
    return _kernel_general(
        inputs, from_tensor, to_tensor, mb, has_mask, has_bias
    )
